# revision 52
# baseline (speedup 1.0000x reference)
"""Self-contained Trainium2 (Bass/Tile) kernel for nn_CQAttention.

kernel(**inputs) takes FULL inputs (B=64) and returns the FULL output
[64, 2048, 512] (= concat[C, A, C*A, C*Bm]). Internally shards batch across
8 NeuronCores (data parallel, 8 batches/core) and runs a Bass/Tile program
via concourse.bass_utils.run_bass_kernel_spmd.

Math (per batch; bias is a constant shift so it cancels in both softmaxes):
  s2[c,q] = sum_d C[c,d]*w4mul[d]*Q[q,d];  s0[c] = C@w4c;  s1[q] = Q@w4q
  G[c,q]  = exp(s0[c] + s2[c,q])     # ONE exp; w4c folded into the moving
                                     # operand so no ACT bias is needed:
                                     # moving[d,q] = Q^T*w4mul + w4c
  S2      = G / colsum_c(G)          # softmax over c (s1 cancels per-q... per
                                     # column factors cancel in the c-softmax)
  M'      = S2^T C                   # via [G^T C | colsum] matmuls (ones col)
  For the q-softmax quantities, the [q,c]-layout matrix needed is EXACTLY
  G^T (bf16 PE transpose): with Qs = Q*exp(s1), es1 = exp(s1),
  Ms = M'*exp(s1)/colsum, the per-c factor exp(-s0[c]) cancels in the
  normalization:
    [numerA | d1 | numerB] = G^T-tiles @ [Qs | es1 | Ms]
    A = numerA / d1,  Bm = numerB / d1
  out = [C, A, C*A, C*Bm]   (C region is written as fp32(bf16(C)); the
  bf16 round-trip is ~4e-3 relative, inside the 2e-2 gate)

Emission is a 3-phase/slot software pipeline across the 8 batches:
  phase1(s): G-matmul+exp(s) | mp(s-1) | loads+cast(s+1)
  phase2(s): ab(s-1)+outputs | q-prep+input-transposes(s+1)
  phase3(s): G-transposes(s) | store(s-1)
so PE fills ACT-gated gaps of G(s) with mp/ab work of batch s-1.
"""
import sys
import numpy as np

for _p in ("/opt/trn_rl_repo",):
    if _p not in sys.path:
        sys.path.insert(0, _p)

import concourse.bass as bass
import concourse.mybir as mybir
import concourse.tile as tile
from concourse import bacc
from concourse.masks import make_identity
from concourse.bass_utils import run_bass_kernel_spmd
from contextlib import ExitStack

F32 = mybir.dt.float32
BF16 = mybir.dt.bfloat16
AF = mybir.ActivationFunctionType
AX = mybir.AxisListType

N_CORES = 8
B, CL, QL, D = 64, 2048, 512, 128
NB = B // N_CORES  # batches per core


def _build_body(nc, tc, ctx, nb, cl, ql, d, C_d, Q_d, w4c_d, w4q_d, w4m_d, OUT_d):
    NT = cl // 128   # 16 c-tiles
    NQ = ql // 128   # 4 q-tiles

    consts = ctx.enter_context(tc.tile_pool(name="consts", bufs=1))
    ident = consts.tile([128, 128], BF16)
    make_identity(nc, ident)
    # [128, d] broadcast rows of the three tiny weight vectors
    w4c_bc = consts.tile([128, d], F32)
    nc.sync.dma_start(w4c_bc, w4c_d.rearrange("d one -> one d")
                      .broadcast_to((128, d)))
    w4q_bc = consts.tile([128, d], F32)
    nc.sync.dma_start(w4q_bc, w4q_d.rearrange("d one -> one d")
                      .broadcast_to((128, d)))
    w4m_bc = consts.tile([128, d], F32)
    nc.sync.dma_start(w4m_bc, w4m_d.rearrange("d one -> one d")
                      .broadcast_to((128, d)))
    ones_bf = consts.tile([128, NT], BF16)
    nc.gpsimd.memset(ones_bf, 1.0)

    # SBUF pools
    ld = ctx.enter_context(tc.tile_pool(name="ld", bufs=3))
    cmp_ = ctx.enter_context(tc.tile_pool(name="cmp", bufs=3))
    qbmp = ctx.enter_context(tc.tile_pool(name="qbmp", bufs=3))
    tpp = ctx.enter_context(tc.tile_pool(name="tpp", bufs=2))
    tmpp = ctx.enter_context(tc.tile_pool(name="tmpp", bufs=2))
    gpool = ctx.enter_context(tc.tile_pool(name="gpool", bufs=2))
    gtp = ctx.enter_context(tc.tile_pool(name="gtp", bufs=2))
    outp = ctx.enter_context(tc.tile_pool(name="outp", bufs=2))
    abst = ctx.enter_context(tc.tile_pool(name="abst", bufs=4))
    stat = ctx.enter_context(tc.tile_pool(name="stat", bufs=2))

    # PSUM pools: g 2 banks x1 + tp 1 bank x2 + acc (mp/ab shared) 1 bank x4
    g_ps = ctx.enter_context(tc.tile_pool(name="g_ps", bufs=1, space="PSUM"))
    t_ps = ctx.enter_context(tc.tile_pool(name="t_ps", bufs=2, space="PSUM"))
    acc_ps = ctx.enter_context(tc.tile_pool(name="acc_ps", bufs=4, space="PSUM"))

    T = {}  # per-batch tile handoff between pipeline stages

    def pre_load(b):
        """HBM loads for batch b (emitted 1 slot ahead)."""
        t = T.setdefault(b, {})
        q_nat = t["q_nat"] = ld.tile([128, NQ, d], F32, tag="q_nat",
                                     name="q_nat")
        nc.sync.dma_start(q_nat, Q_d[b].rearrange("(t p) d -> p t d", p=128))
        c_nat = t["c_nat"] = ld.tile([128, NT, d], F32, tag="c_nat",
                                     name="c_nat")
        c_r = C_d[b].rearrange("(t p) d -> p t d", p=128)
        nc.sync.dma_start(c_nat[:, 0:8, :], c_r[:, 0:8, :])
        nc.sync.dma_start(c_nat[:, 8:NT, :], c_r[:, 8:NT, :])
        yield

    def pre_cast(b):
        """Pool: cm = [bf16(C) | ones]."""
        t = T[b]
        c_nat = t["c_nat"]
        cm = t["cm"] = cmp_.tile([128, NT, d + 1], BF16, tag="cm", name="cm")
        for g in range(4):
            nc.gpsimd.tensor_copy(cm[:, g * 4:(g + 1) * 4, 0:d],
                                  c_nat[:, g * 4:(g + 1) * 4, :])
            yield
        nc.gpsimd.tensor_copy(cm[:, :, d], ones_bf[:, 0:NT])
        yield

    def pre_q(b):
        """DVE/ACT/PE: s1, es1, qbm(Qs|es1|..|es1), ct, qwt2."""
        t = T[b]
        while "q_nat" not in t or "cm" not in t:
            yield
        q_nat, cm = t["q_nat"], t["cm"]
        qm2a = tmpp.tile([128, NQ, d], BF16, tag="qm2a")
        nc.vector.tensor_mul(qm2a, q_nat,
                             w4m_bc.unsqueeze(1).broadcast_to((128, NQ, d)))
        qm2 = tmpp.tile([128, NQ, d], BF16, tag="qm2")
        nc.vector.tensor_add(qm2, qm2a,
                             w4c_bc.unsqueeze(1).broadcast_to((128, NQ, d)))
        yield
        qwt2 = t["qwt2"] = tpp.tile([128, NQ, d], BF16, tag="qwt2",
                                    name="qwt2")
        tpq = t_ps.tile([128, NQ, 128], BF16, tag="tp", name="tpq")
        for i in range(NQ):
            nc.tensor.transpose(tpq[:, i, :], qm2[:, i, :], ident)
        nc.vector.tensor_copy(qwt2, tpq)
        yield
        # s1 row-dot
        tmq = tmpp.tile([128, NQ, d], BF16, tag="tmq")
        nc.vector.tensor_mul(tmq, q_nat,
                             w4q_bc.unsqueeze(1).broadcast_to((128, NQ, d)))
        s1_pt = stat.tile([128, NQ], F32, tag="s1")
        nc.vector.reduce_sum(s1_pt, tmq, axis=AX.X)
        yield
        es1 = t["es1"] = stat.tile([128, NQ], F32, tag="es1", name="es1")
        nc.scalar.activation(es1, s1_pt, AF.Exp)
        # moving operand layout: [Qs(0:d) | Ms(d:2d) | es1 | es1] — keeps every
        # bf16 slice at an even element offset (packed-pair alignment)
        qbm = t["qbm"] = qbmp.tile([128, NQ, 2 * d + 2], BF16, tag="qbm",
                                   name="qbm")
        nc.vector.tensor_copy(qbm[:, :, 2 * d:2 * d + 2],
                              es1.unsqueeze(2).broadcast_to((128, NQ, 2)))
        yield
        nc.vector.tensor_mul(qbm[:, :, 0:d], q_nat,
                             es1.unsqueeze(2).broadcast_to((128, NQ, d)))
        yield
        ct = t["ct"] = tpp.tile([128, NT, d], BF16, tag="ct", name="ct")
        for g in range(NT // 4):
            tp = t_ps.tile([128, 4, 128], BF16, tag="tp", name="tpc")
            for i in range(4):
                nc.tensor.transpose(tp[:, i, :], cm[:, g * 4 + i, 0:d], ident)
            nc.vector.tensor_copy(ct[:, g * 4:(g + 1) * 4, :], tp)
            yield

    def g_units(b):
        """PE: G-matmuls; ACT: exp -> G bf16 [c, 16, 512]."""
        t = T[b]
        ct, qwt2 = t["ct"], t["qwt2"]
        qwt2_flat = qwt2.rearrange("p a b -> p (a b)")
        G = t["G"] = gpool.tile([128, NT, ql], BF16, tag="G", name="G")
        for p in range(NT // 2):
            gps = g_ps.tile([128, 2, ql], F32, tag="gps", name="gps")
            nc.tensor.matmul(gps[:, 0, :], ct[:, 2 * p, :], qwt2_flat)
            nc.tensor.matmul(gps[:, 1, :], ct[:, 2 * p + 1, :], qwt2_flat)
            nc.scalar.activation(G[:, 2 * p:2 * p + 2, :], gps, AF.Exp)
            t["g_done"] = p + 1
            yield

    def mp_units(b):
        """PE: [M'raw | colsum] accumulation; DVE: Ms -> qbm[129:257]."""
        t = T[b]
        G, cm, qbm, es1 = t["G"], t["cm"], t["qbm"], t["es1"]
        rcol = stat.tile([128, NQ], F32, tag="rcol")
        rs = stat.tile([128, NQ], F32, tag="rs")
        mpps = None
        for qi in range(NQ):
            if qi % 2 == 0:
                mpps = acc_ps.tile([128, 2, d + 1], F32, tag="acc",
                                   name="mpps")
            g = qi % 2
            for tt in range(NT):
                nc.tensor.matmul(mpps[:, g, :],
                                 G[:, tt, qi * 128:(qi + 1) * 128],
                                 cm[:, tt, :],
                                 start=(tt == 0), stop=(tt == NT - 1))
            if g == 1:
                q0 = qi - 1
                nc.vector.reciprocal(rcol[:, q0:qi + 1], mpps[:, :, d])
                nc.vector.tensor_mul(rs[:, q0:qi + 1], rcol[:, q0:qi + 1],
                                     es1[:, q0:qi + 1])
                for qj in (q0, qi):
                    nc.vector.tensor_scalar_mul(qbm[:, qj, d:2 * d],
                                                mpps[:, qj % 2, 0:d],
                                                rs[:, qj:qj + 1])
                    t["ms_done"] = qj + 1
            yield

    def t_units(b):
        """PE: 64 transposes of G -> gt bf16 [q, 4, 16, 128]; DVE drains
        (8 tiles per drain so the SBUF/PSUM access bubble amortizes)."""
        t = T[b]
        G = t["G"]
        gt = t["gt"] = gtp.tile([128, NQ, NT, 128], BF16, tag="gt", name="gt")
        for c8 in range(NT // 8):
            # emission-order: wait for the exps covering this c-block
            while t.get("g_done", 0) < (c8 + 1) * 4:
                yield
            for qi in range(NQ):
                tps = t_ps.tile([128, 8, 128], BF16, tag="tp", name="tpg")
                for k in range(8):
                    tt = c8 * 8 + k
                    nc.tensor.transpose(tps[:, k, :],
                                        G[:, tt, qi * 128:(qi + 1) * 128],
                                        ident)
                nc.vector.tensor_copy(gt[:, qi, c8 * 8:(c8 + 1) * 8, :], tps)
                yield

    def ab_units(b):
        """PE: fused [numerA|d1|numerB] matmuls; drains alternate ACT/DVE;
        DVE: 1/d1 + A; Pool: fused (numer*rd1)*C for C*A and C*Bm."""
        t = T[b]
        # emission-order hazard: the fused matmul reads qbm's Ms columns,
        # which mp_units of the SAME batch emits concurrently in the
        # round-robin — reads emitted before writes would bind to stale data.
        while t.get("ms_done", 0) < NQ:
            yield
        gt, qbm = t["gt"], t["qbm"]
        c_nat = t["c_nat"]
        outb = t["outb"] = outp.tile([128, NT, 4 * d], F32, tag="outb",
                                     name="outb")
        nc.vector.tensor_copy(outb[:, :, 0:d], c_nat)
        rrow = stat.tile([128, NT], F32, tag="rrow")
        out_r = OUT_d[b].rearrange("(t p) n -> p t n", p=128)
        abr = None
        for tt in range(NT):
            if tt % 2 == 0:
                abr = abst.tile([128, 2, 2 * d + 2], BF16, tag="abr",
                                name="abr")
                bm2 = abst.tile([128, 2, d], BF16, tag="bm2", name="bm2")
            j = tt % 2
            abt = acc_ps.tile([128, 2, d + 1], F32, tag="acc", name="abps")
            abps = abt.rearrange("p a b -> p (a b)")
            for qi in range(NQ):
                nc.tensor.matmul(abps, gt[:, qi, tt, :], qbm[:, qi, :],
                                 start=(qi == 0), stop=(qi == NQ - 1))
            nc.scalar.activation(abr[:, j, :], abps, AF.Copy)
            if j == 1:
                nc.vector.reciprocal(rrow[:, tt - 1:tt + 1], abr[:, :, 2 * d])
                for k in range(2):
                    t2 = tt - 1 + k
                    r = rrow[:, t2:t2 + 1]
                    nc.vector.tensor_scalar_mul(outb[:, t2, d:2 * d],
                                                abr[:, k, 0:d], r)
                    nc.vector.tensor_scalar_mul(bm2[:, k, :],
                                                abr[:, k, d:2 * d], r)
                cn2 = c_nat[:, tt - 1:tt + 1, :]
                nc.gpsimd.tensor_mul(outb[:, tt - 1:tt + 1, 2 * d:3 * d],
                                     outb[:, tt - 1:tt + 1, d:2 * d], cn2)
                nc.gpsimd.tensor_mul(outb[:, tt - 1:tt + 1, 3 * d:4 * d],
                                     bm2, cn2)
            if tt % 4 == 3:
                qtr = tt // 4
                nc.sync.dma_start(out_r[:, qtr * 4:(qtr + 1) * 4, :],
                                  outb[:, qtr * 4:(qtr + 1) * 4, :])
            yield
        del T[b]

    def run_rr(gens):
        live = list(gens)
        while live:
            for g in list(live):
                try:
                    next(g)
                except StopIteration:
                    live.remove(g)

    # preamble: batch 0 prep
    run_rr([pre_load(0)])
    run_rr([pre_cast(0), pre_q(0)])

    for s in range(nb + 1):
        g1 = []
        if s < nb:
            g1.append(g_units(s))
        if s >= 1:
            g1.append(mp_units(s - 1))
            g1.append(ab_units(s - 1))
        if s < nb:
            g1.append(t_units(s))
        if s + 1 < nb:
            g1.append(pre_load(s + 1))
            g1.append(pre_cast(s + 1))
            g1.append(pre_q(s + 1))
        run_rr(g1)


def build_program(nb=NB):
    nc = bacc.Bacc("TRN2", target_bir_lowering=False, debug=False,
                   num_devices=N_CORES)
    C_d = nc.dram_tensor("C", [nb, CL, D], F32, kind="ExternalInput").ap()
    Q_d = nc.dram_tensor("Q", [nb, QL, D], F32, kind="ExternalInput").ap()
    w4c_d = nc.dram_tensor("w4c", [D, 1], F32, kind="ExternalInput").ap()
    w4q_d = nc.dram_tensor("w4q", [D, 1], F32, kind="ExternalInput").ap()
    w4m_d = nc.dram_tensor("w4mul", [D, 1], F32, kind="ExternalInput").ap()
    OUT_d = nc.dram_tensor("OUT", [nb, CL, 4 * D], F32, kind="ExternalOutput").ap()
    with ExitStack() as ctx:
        tc = ctx.enter_context(tile.TileContext(nc))
        _build_body(nc, tc, ctx, nb, CL, QL, D,
                    C_d, Q_d, w4c_d, w4q_d, w4m_d, OUT_d)
    nc.compile()
    return nc


_PROGRAM_CACHE = {}


def _get_program(nb=NB):
    if nb not in _PROGRAM_CACHE:
        _PROGRAM_CACHE[nb] = build_program(nb)
    return _PROGRAM_CACHE[nb]


def _numpy_fallback(C, Q, c_mask, q_mask, w4c, w4q, w4mul, bias):
    """Exact reference math in numpy (used only if masks are not all-ones)."""
    NEG_INF = -1e30
    out = np.empty((C.shape[0], C.shape[1], 4 * C.shape[2]), np.float32)
    for b in range(C.shape[0]):
        Cb = C[b].astype(np.float64)
        Qb = Q[b].astype(np.float64)
        S = (Cb @ w4c.reshape(-1, 1) + (Qb @ w4q.reshape(-1, 1)).T
             + (Cb * w4mul.reshape(1, -1)) @ Qb.T + float(np.asarray(bias).reshape(-1)[0]))
        qm = q_mask[b].reshape(1, -1)
        cm = c_mask[b].reshape(-1, 1)
        S1l = S * qm + NEG_INF * (1.0 - qm)
        S2l = S * cm + NEG_INF * (1.0 - cm)
        S1 = np.exp(S1l - S1l.max(1, keepdims=True))
        S1 /= S1.sum(1, keepdims=True)
        S2 = np.exp(S2l - S2l.max(0, keepdims=True))
        S2 /= S2.sum(0, keepdims=True)
        A = S1 @ Qb
        Bm = S1 @ (S2.T @ Cb)
        out[b] = np.concatenate([Cb, A, Cb * A, Cb * Bm], axis=1).astype(np.float32)
    return out


def kernel(C, Q, c_mask, q_mask, w4c, w4q, w4mul, bias):
    C = np.ascontiguousarray(np.asarray(C), dtype=np.float32)
    Q = np.ascontiguousarray(np.asarray(Q), dtype=np.float32)
    c_mask = np.asarray(c_mask)
    q_mask = np.asarray(q_mask)
    w4c = np.asarray(w4c, dtype=np.float32)
    w4q = np.asarray(w4q, dtype=np.float32)
    w4mul = np.asarray(w4mul, dtype=np.float32)

    if not (np.all(c_mask == 1.0) and np.all(q_mask == 1.0)):
        return _numpy_fallback(C, Q, c_mask, q_mask, w4c, w4q, w4mul, bias)

    nc = _get_program(NB)
    w4c_r = np.ascontiguousarray(w4c.reshape(D, 1))
    w4q_r = np.ascontiguousarray(w4q.reshape(D, 1))
    w4m_r = np.ascontiguousarray(w4mul.reshape(D, 1))
    in_maps = []
    for c in range(N_CORES):
        sl = slice(c * NB, (c + 1) * NB)
        in_maps.append({
            "C": np.ascontiguousarray(C[sl]),
            "Q": np.ascontiguousarray(Q[sl]),
            "w4c": w4c_r,
            "w4q": w4q_r,
            "w4mul": w4m_r,
        })
    res = run_bass_kernel_spmd(nc, in_maps, core_ids=list(range(N_CORES)))
    out = np.concatenate([res.results[c]["OUT"] for c in range(N_CORES)], axis=0)
    return out


# revision 54
# speedup vs baseline: 1.1318x; 1.1318x over previous
"""Self-contained Trainium2 (Bass/Tile) kernel for nn_CQAttention.

kernel(**inputs) takes FULL inputs (B=64) and returns the FULL output
[64, 2048, 512] (= concat[C, A, C*A, C*Bm]). Internally shards batch across
8 NeuronCores (data parallel, 8 batches/core) and runs a Bass/Tile program
via concourse.bass_utils.run_bass_kernel_spmd.

Math (per batch; bias is a constant shift so it cancels in both softmaxes):
  s2[c,q] = sum_d C[c,d]*w4mul[d]*Q[q,d];  s0[c] = C@w4c;  s1[q] = Q@w4q
  G[c,q]  = exp(s0[c] + s2[c,q])     # ONE exp; w4c folded into the moving
                                     # operand so no ACT bias is needed:
                                     # moving[d,q] = Q^T*w4mul + w4c
  S2      = G / colsum_c(G)          # softmax over c (s1 cancels per-q... per
                                     # column factors cancel in the c-softmax)
  M'      = S2^T C                   # via [G^T C | colsum] matmuls (ones col)
  For the q-softmax quantities, the [q,c]-layout matrix needed is EXACTLY
  G^T (bf16 PE transpose): with Qs = Q*exp(s1), es1 = exp(s1),
  Ms = M'*exp(s1)/colsum, the per-c factor exp(-s0[c]) cancels in the
  normalization:
    [numerA | d1 | numerB] = G^T-tiles @ [Qs | es1 | Ms]
    A = numerA / d1,  Bm = numerB / d1
  out = [C, A, C*A, C*Bm]   (C region is written as fp32(bf16(C)); the
  bf16 round-trip is ~4e-3 relative, inside the 2e-2 gate)

Emission is a 3-phase/slot software pipeline across the 8 batches:
  phase1(s): G-matmul+exp(s) | mp(s-1) | loads+cast(s+1)
  phase2(s): ab(s-1)+outputs | q-prep+input-transposes(s+1)
  phase3(s): G-transposes(s) | store(s-1)
so PE fills ACT-gated gaps of G(s) with mp/ab work of batch s-1.
"""
import sys
import numpy as np

for _p in ("/opt/trn_rl_repo",):
    if _p not in sys.path:
        sys.path.insert(0, _p)

import concourse.bass as bass
import concourse.mybir as mybir
import concourse.tile as tile
from concourse import bacc
from concourse.masks import make_identity
from concourse.bass_utils import run_bass_kernel_spmd
from contextlib import ExitStack

F32 = mybir.dt.float32
BF16 = mybir.dt.bfloat16
AF = mybir.ActivationFunctionType
AX = mybir.AxisListType

N_CORES = 8
B, CL, QL, D = 64, 2048, 512, 128
NB = B // N_CORES  # batches per core


def _build_body(nc, tc, ctx, nb, cl, ql, d, C_d, Q_d, w4c_d, w4q_d, w4m_d, OUT_d):
    NT = cl // 128   # 16 c-tiles
    NQ = ql // 128   # 4 q-tiles

    consts = ctx.enter_context(tc.tile_pool(name="consts", bufs=1))
    ident = consts.tile([128, 128], BF16)
    make_identity(nc, ident)
    # [128, d] broadcast rows of the three tiny weight vectors
    w4c_bc = consts.tile([128, d], F32)
    nc.sync.dma_start(w4c_bc, w4c_d.rearrange("d one -> one d")
                      .broadcast_to((128, d)))
    w4q_bc = consts.tile([128, d], F32)
    nc.sync.dma_start(w4q_bc, w4q_d.rearrange("d one -> one d")
                      .broadcast_to((128, d)))
    w4m_bc = consts.tile([128, d], F32)
    nc.sync.dma_start(w4m_bc, w4m_d.rearrange("d one -> one d")
                      .broadcast_to((128, d)))
    ones_bf = consts.tile([128, NT], BF16)
    nc.gpsimd.memset(ones_bf, 1.0)

    # SBUF pools
    ld = ctx.enter_context(tc.tile_pool(name="ld", bufs=3))
    cmp_ = ctx.enter_context(tc.tile_pool(name="cmp", bufs=3))
    qbmp = ctx.enter_context(tc.tile_pool(name="qbmp", bufs=3))
    tpp = ctx.enter_context(tc.tile_pool(name="tpp", bufs=2))
    tmpp = ctx.enter_context(tc.tile_pool(name="tmpp", bufs=2))
    gpool = ctx.enter_context(tc.tile_pool(name="gpool", bufs=2))
    gtp = ctx.enter_context(tc.tile_pool(name="gtp", bufs=2))
    outp = ctx.enter_context(tc.tile_pool(name="outp", bufs=2))
    abst = ctx.enter_context(tc.tile_pool(name="abst", bufs=4))
    stat = ctx.enter_context(tc.tile_pool(name="stat", bufs=2))

    # PSUM pools: g 2 banks x1 + tp 1 bank x2 + acc (mp/ab shared) 1 bank x4
    g_ps = ctx.enter_context(tc.tile_pool(name="g_ps", bufs=1, space="PSUM"))
    t_ps = ctx.enter_context(tc.tile_pool(name="t_ps", bufs=2, space="PSUM"))
    acc_ps = ctx.enter_context(tc.tile_pool(name="acc_ps", bufs=4, space="PSUM"))

    T = {}  # per-batch tile handoff between pipeline stages

    def pre_load(b):
        """HBM loads for batch b (emitted 1 slot ahead)."""
        t = T.setdefault(b, {})
        q_nat = t["q_nat"] = ld.tile([128, NQ, d], F32, tag="q_nat",
                                     name="q_nat")
        nc.sync.dma_start(q_nat, Q_d[b].rearrange("(t p) d -> p t d", p=128))
        c_nat = t["c_nat"] = ld.tile([128, NT, d], F32, tag="c_nat",
                                     name="c_nat")
        c_r = C_d[b].rearrange("(t p) d -> p t d", p=128)
        nc.sync.dma_start(c_nat[:, 0:8, :], c_r[:, 0:8, :])
        nc.sync.dma_start(c_nat[:, 8:NT, :], c_r[:, 8:NT, :])
        yield

    def pre_cast(b):
        """Pool: cm = [bf16(C) | ones]."""
        t = T[b]
        c_nat = t["c_nat"]
        cm = t["cm"] = cmp_.tile([128, NT, d + 1], BF16, tag="cm", name="cm")
        for g in range(4):
            with tc.high_priority(offset=400):
                nc.gpsimd.tensor_copy(cm[:, g * 4:(g + 1) * 4, 0:d],
                                      c_nat[:, g * 4:(g + 1) * 4, :])
            yield
        nc.gpsimd.tensor_copy(cm[:, :, d], ones_bf[:, 0:NT])
        yield

    def pre_q(b):
        """DVE/ACT/PE: s1, es1, qbm(Qs|es1|..|es1), ct, qwt2."""
        t = T[b]
        while "q_nat" not in t or "cm" not in t:
            yield
        q_nat, cm = t["q_nat"], t["cm"]
        qm2a = tmpp.tile([128, NQ, d], BF16, tag="qm2a")
        nc.vector.tensor_mul(qm2a, q_nat,
                             w4m_bc.unsqueeze(1).broadcast_to((128, NQ, d)))
        qm2 = tmpp.tile([128, NQ, d], BF16, tag="qm2")
        nc.vector.tensor_add(qm2, qm2a,
                             w4c_bc.unsqueeze(1).broadcast_to((128, NQ, d)))
        yield
        qwt2 = t["qwt2"] = tpp.tile([128, NQ, d], BF16, tag="qwt2",
                                    name="qwt2")
        tpq = t_ps.tile([128, NQ, 128], BF16, tag="tp", name="tpq")
        for i in range(NQ):
            nc.tensor.transpose(tpq[:, i, :], qm2[:, i, :], ident)
        nc.vector.tensor_copy(qwt2, tpq)
        yield
        # s1 row-dot
        tmq = tmpp.tile([128, NQ, d], BF16, tag="tmq")
        nc.vector.tensor_mul(tmq, q_nat,
                             w4q_bc.unsqueeze(1).broadcast_to((128, NQ, d)))
        s1_pt = stat.tile([128, NQ], F32, tag="s1")
        nc.vector.reduce_sum(s1_pt, tmq, axis=AX.X)
        yield
        es1 = t["es1"] = stat.tile([128, NQ], F32, tag="es1", name="es1")
        nc.scalar.activation(es1, s1_pt, AF.Exp)
        # moving operand layout: [Qs(0:d) | Ms(d:2d) | es1 | es1] — keeps every
        # bf16 slice at an even element offset (packed-pair alignment)
        qbm = t["qbm"] = qbmp.tile([128, NQ, 2 * d + 2], BF16, tag="qbm",
                                   name="qbm")
        nc.vector.tensor_copy(qbm[:, :, 2 * d:2 * d + 2],
                              es1.unsqueeze(2).broadcast_to((128, NQ, 2)))
        yield
        nc.vector.tensor_mul(qbm[:, :, 0:d], q_nat,
                             es1.unsqueeze(2).broadcast_to((128, NQ, d)))
        yield
        ct = t["ct"] = tpp.tile([128, NT, d], BF16, tag="ct", name="ct")
        for g in range(NT // 4):
            tp = t_ps.tile([128, 4, 128], BF16, tag="tp", name="tpc")
            for i in range(4):
                nc.tensor.transpose(tp[:, i, :], cm[:, g * 4 + i, 0:d], ident)
            nc.vector.tensor_copy(ct[:, g * 4:(g + 1) * 4, :], tp)
            yield

    def g_units(b):
        """PE: G-matmuls; ACT: exp -> G bf16 [c, 16, 512]."""
        t = T[b]
        ct, qwt2 = t["ct"], t["qwt2"]
        qwt2_flat = qwt2.rearrange("p a b -> p (a b)")
        G = t["G"] = gpool.tile([128, NT, ql], BF16, tag="G", name="G")
        for p in range(NT // 2):
            gps = g_ps.tile([128, 2, ql], F32, tag="gps", name="gps")
            nc.tensor.matmul(gps[:, 0, :], ct[:, 2 * p, :], qwt2_flat)
            nc.tensor.matmul(gps[:, 1, :], ct[:, 2 * p + 1, :], qwt2_flat)
            nc.scalar.activation(G[:, 2 * p:2 * p + 2, :], gps, AF.Exp)
            t["g_done"] = p + 1
            yield

    def mp_units(b):
        """PE: [M'raw | colsum] accumulation; DVE: Ms -> qbm[129:257]."""
        t = T[b]
        G, cm, qbm, es1 = t["G"], t["cm"], t["qbm"], t["es1"]
        rcol = stat.tile([128, NQ], F32, tag="rcol")
        rs = stat.tile([128, NQ], F32, tag="rs")
        mpps = None
        for qi in range(NQ):
            if qi % 2 == 0:
                mpps = acc_ps.tile([128, 2, d + 1], F32, tag="acc",
                                   name="mpps")
            g = qi % 2
            for tt in range(NT):
                nc.tensor.matmul(mpps[:, g, :],
                                 G[:, tt, qi * 128:(qi + 1) * 128],
                                 cm[:, tt, :],
                                 start=(tt == 0), stop=(tt == NT - 1))
            if g == 1:
                q0 = qi - 1
                nc.vector.reciprocal(rcol[:, q0:qi + 1], mpps[:, :, d])
                nc.vector.tensor_mul(rs[:, q0:qi + 1], rcol[:, q0:qi + 1],
                                     es1[:, q0:qi + 1])
                for qj in (q0, qi):
                    nc.vector.tensor_scalar_mul(qbm[:, qj, d:2 * d],
                                                mpps[:, qj % 2, 0:d],
                                                rs[:, qj:qj + 1])
                    t["ms_done"] = qj + 1
            yield

    def t_units(b):
        """PE: 64 transposes of G -> gt bf16 [q, 4, 16, 128]; DVE drains
        (8 tiles per drain so the SBUF/PSUM access bubble amortizes)."""
        t = T[b]
        G = t["G"]
        gt = t["gt"] = gtp.tile([128, NQ, NT, 128], BF16, tag="gt", name="gt")
        for c8 in range(NT // 8):
            # emission-order: wait for the exps covering this c-block
            while t.get("g_done", 0) < (c8 + 1) * 4:
                yield
            for qi in range(NQ):
                tps = t_ps.tile([128, 8, 128], BF16, tag="tp", name="tpg")
                for k in range(8):
                    tt = c8 * 8 + k
                    nc.tensor.transpose(tps[:, k, :],
                                        G[:, tt, qi * 128:(qi + 1) * 128],
                                        ident)
                nc.vector.tensor_copy(gt[:, qi, c8 * 8:(c8 + 1) * 8, :], tps)
                yield

    def ab_units(b):
        """PE: fused [numerA|d1|numerB] matmuls; drains alternate ACT/DVE;
        DVE: 1/d1 + A; Pool: fused (numer*rd1)*C for C*A and C*Bm."""
        t = T[b]
        # emission-order hazard: the fused matmul reads qbm's Ms columns,
        # which mp_units of the SAME batch emits concurrently in the
        # round-robin — reads emitted before writes would bind to stale data.
        while t.get("ms_done", 0) < NQ:
            yield
        gt, qbm = t["gt"], t["qbm"]
        c_nat = t["c_nat"]
        outb = t["outb"] = outp.tile([128, NT, 4 * d], F32, tag="outb",
                                     name="outb")
        nc.vector.tensor_copy(outb[:, :, 0:d], c_nat)
        rrow = stat.tile([128, NT], F32, tag="rrow")
        out_r = OUT_d[b].rearrange("(t p) n -> p t n", p=128)
        abr = None
        for tt in range(NT):
            if tt % 2 == 0:
                abr = abst.tile([128, 2, 2 * d + 2], BF16, tag="abr",
                                name="abr")
                bm2 = abst.tile([128, 2, d], BF16, tag="bm2", name="bm2")
            j = tt % 2
            abt = acc_ps.tile([128, 2, d + 1], F32, tag="acc", name="abps")
            abps = abt.rearrange("p a b -> p (a b)")
            for qi in range(NQ):
                nc.tensor.matmul(abps, gt[:, qi, tt, :], qbm[:, qi, :],
                                 start=(qi == 0), stop=(qi == NQ - 1))
            with tc.high_priority(offset=400):
                nc.scalar.activation(abr[:, j, :], abps, AF.Copy)
            if j == 1:
                nc.vector.reciprocal(rrow[:, tt - 1:tt + 1], abr[:, :, 2 * d])
                for k in range(2):
                    t2 = tt - 1 + k
                    r = rrow[:, t2:t2 + 1]
                    nc.vector.tensor_scalar_mul(outb[:, t2, d:2 * d],
                                                abr[:, k, 0:d], r)
                    nc.vector.tensor_scalar_mul(bm2[:, k, :],
                                                abr[:, k, d:2 * d], r)
                cn2 = c_nat[:, tt - 1:tt + 1, :]
                # last batch: Pool is the tail's critical chain; use the
                # otherwise-idle DVE for the output products instead
                eng = nc.vector if b == nb - 1 else nc.gpsimd
                eng.tensor_mul(outb[:, tt - 1:tt + 1, 2 * d:3 * d],
                               outb[:, tt - 1:tt + 1, d:2 * d], cn2)
                eng.tensor_mul(outb[:, tt - 1:tt + 1, 3 * d:4 * d],
                               bm2, cn2)
            if tt % 4 == 3:
                qtr = tt // 4
                nc.sync.dma_start(out_r[:, qtr * 4:(qtr + 1) * 4, :],
                                  outb[:, qtr * 4:(qtr + 1) * 4, :])
            yield
        del T[b]

    def run_rr(gens):
        live = list(gens)
        while live:
            for g in list(live):
                try:
                    next(g)
                except StopIteration:
                    live.remove(g)

    # preamble: batch 0 prep
    run_rr([pre_load(0)])
    run_rr([pre_cast(0), pre_q(0)])

    for s in range(nb + 1):
        g1 = []
        if s < nb:
            g1.append(g_units(s))
        if s >= 1:
            g1.append(mp_units(s - 1))
            g1.append(ab_units(s - 1))
        if s < nb:
            g1.append(t_units(s))
        if s + 1 < nb:
            g1.append(pre_load(s + 1))
            g1.append(pre_cast(s + 1))
            g1.append(pre_q(s + 1))
        run_rr(g1)


def build_program(nb=NB):
    nc = bacc.Bacc("TRN2", target_bir_lowering=False, debug=False,
                   num_devices=N_CORES)
    C_d = nc.dram_tensor("C", [nb, CL, D], F32, kind="ExternalInput").ap()
    Q_d = nc.dram_tensor("Q", [nb, QL, D], F32, kind="ExternalInput").ap()
    w4c_d = nc.dram_tensor("w4c", [D, 1], F32, kind="ExternalInput").ap()
    w4q_d = nc.dram_tensor("w4q", [D, 1], F32, kind="ExternalInput").ap()
    w4m_d = nc.dram_tensor("w4mul", [D, 1], F32, kind="ExternalInput").ap()
    OUT_d = nc.dram_tensor("OUT", [nb, CL, 4 * D], F32, kind="ExternalOutput").ap()
    with ExitStack() as ctx:
        tc = ctx.enter_context(tile.TileContext(nc))
        _build_body(nc, tc, ctx, nb, CL, QL, D,
                    C_d, Q_d, w4c_d, w4q_d, w4m_d, OUT_d)
    nc.compile()
    return nc


_PROGRAM_CACHE = {}


def _get_program(nb=NB):
    if nb not in _PROGRAM_CACHE:
        _PROGRAM_CACHE[nb] = build_program(nb)
    return _PROGRAM_CACHE[nb]


def _numpy_fallback(C, Q, c_mask, q_mask, w4c, w4q, w4mul, bias):
    """Exact reference math in numpy (used only if masks are not all-ones)."""
    NEG_INF = -1e30
    out = np.empty((C.shape[0], C.shape[1], 4 * C.shape[2]), np.float32)
    for b in range(C.shape[0]):
        Cb = C[b].astype(np.float64)
        Qb = Q[b].astype(np.float64)
        S = (Cb @ w4c.reshape(-1, 1) + (Qb @ w4q.reshape(-1, 1)).T
             + (Cb * w4mul.reshape(1, -1)) @ Qb.T + float(np.asarray(bias).reshape(-1)[0]))
        qm = q_mask[b].reshape(1, -1)
        cm = c_mask[b].reshape(-1, 1)
        S1l = S * qm + NEG_INF * (1.0 - qm)
        S2l = S * cm + NEG_INF * (1.0 - cm)
        S1 = np.exp(S1l - S1l.max(1, keepdims=True))
        S1 /= S1.sum(1, keepdims=True)
        S2 = np.exp(S2l - S2l.max(0, keepdims=True))
        S2 /= S2.sum(0, keepdims=True)
        A = S1 @ Qb
        Bm = S1 @ (S2.T @ Cb)
        out[b] = np.concatenate([Cb, A, Cb * A, Cb * Bm], axis=1).astype(np.float32)
    return out


def kernel(C, Q, c_mask, q_mask, w4c, w4q, w4mul, bias):
    C = np.ascontiguousarray(np.asarray(C), dtype=np.float32)
    Q = np.ascontiguousarray(np.asarray(Q), dtype=np.float32)
    c_mask = np.asarray(c_mask)
    q_mask = np.asarray(q_mask)
    w4c = np.asarray(w4c, dtype=np.float32)
    w4q = np.asarray(w4q, dtype=np.float32)
    w4mul = np.asarray(w4mul, dtype=np.float32)

    if not (np.all(c_mask == 1.0) and np.all(q_mask == 1.0)):
        return _numpy_fallback(C, Q, c_mask, q_mask, w4c, w4q, w4mul, bias)

    nc = _get_program(NB)
    w4c_r = np.ascontiguousarray(w4c.reshape(D, 1))
    w4q_r = np.ascontiguousarray(w4q.reshape(D, 1))
    w4m_r = np.ascontiguousarray(w4mul.reshape(D, 1))
    in_maps = []
    for c in range(N_CORES):
        sl = slice(c * NB, (c + 1) * NB)
        in_maps.append({
            "C": np.ascontiguousarray(C[sl]),
            "Q": np.ascontiguousarray(Q[sl]),
            "w4c": w4c_r,
            "w4q": w4q_r,
            "w4mul": w4m_r,
        })
    res = run_bass_kernel_spmd(nc, in_maps, core_ids=list(range(N_CORES)))
    out = np.concatenate([res.results[c]["OUT"] for c in range(N_CORES)], axis=0)
    return out
